# revision 1
# baseline (speedup 1.0000x reference)
"""MultiHeadPool Trainium2 kernel.

Per-core computation (batch b of 8, one per NeuronCore):
  X = others[b]          (N=64, T=512, D=128)
  L = X . qT * scale     contraction over d   -> (T, H, N) logits
  W = softmax_n(L)
  ctx = W . X            contraction over n   -> (T, H, D)

Layout: t-pairs stacked on partitions. Host pre-arranges input as
xq[(j n), tp, d'] = others[n, 2 tp + j, d] with d'==D a ones column, so each
chunk is one contiguous DMA and mm2 emits softmax denominators for free.

Per t-pair tile S_c = (128 = [t-even n's; t-odd n's], 129):
  1. PE transpose-mode matmul: X^T_c = S_c.T          (PSUM)
  2. relay X^T_c -> SBUF (DVE / ACT alternating)
  3. mm1: L^T_c = X_c @ qT_scaled                     ((j,n), 7) PSUM
  4. exp on ACT from PSUM into block-diagonal E tiles (eps-padded to M=32)
  5. mm2: ctx-pair = E_blk.T @ S_c -> (32, 129) at col-group 32*g2;
     col 128 = denominators; 4 col-groups run concurrently on the PE array
  6. DVE: reciprocal + broadcast scale; DMA out
"""

import sys

for p in ("/opt/trn_rl_repo", "/root/.axon_site/_ro/trn_rl_repo"):
    if p not in sys.path:
        sys.path.append(p)

from contextlib import ExitStack

import numpy as np

import concourse.bacc as bacc
import concourse.bass as bass
import concourse.tile as tile
from concourse import mybir
from concourse.bass_utils import run_bass_kernel_spmd
from concourse.tile import add_dep_helper

B, N, T, D, H = 8, 64, 512, 128, 7
CH = 16               # t-pairs per DMA chunk
NG = (T // 2) // CH   # 16 chunks per batch
F32 = mybir.dt.float32

_CACHE = {}


def _body(ctx, tc, xb, qt, ident, ob, repeat=1):
    nc = tc.nc
    _body.prev_st = None
    E2 = 2 * H            # 14
    M2 = 32               # mm2 stationary cols (14 data + 18 eps pad)
    DE = D + 1            # 129

    singles = ctx.enter_context(tc.tile_pool(name="singles", bufs=1))
    chunks = ctx.enter_context(tc.tile_pool(name="chunks", bufs=3))
    xtp = ctx.enter_context(tc.tile_pool(name="xtp", bufs=4, space="PSUM"))
    xts = ctx.enter_context(tc.tile_pool(name="xts", bufs=3))
    ltp = ctx.enter_context(tc.tile_pool(name="ltp", bufs=1, space="PSUM"))
    ep = ctx.enter_context(tc.tile_pool(name="ep", bufs=2))
    ctxp = ctx.enter_context(tc.tile_pool(name="ctxp", bufs=3, space="PSUM"))
    rp = ctx.enter_context(tc.tile_pool(name="rp", bufs=4))
    stg = ctx.enter_context(tc.tile_pool(name="stg", bufs=3))

    # fp32 matmuls are single fused instructions with ONE sync-wait slot in
    # the ISA; PE nops (via add_dep_helper) absorb every cross-engine wait
    # before the matmuls that would otherwise collect them.
    def pe_absorb(ap):
        # a dummy bf16 weight-load that "reads" the tile: a real tracked PE
        # instruction with no output, so it collects the cross-engine wait
        # instead of the next matmul (fused fp32 form has ONE wait slot).
        # The loaded junk weights are irrelevant: fp32 matmuls self-load.
        nc.tensor.ldweights(weights=ap.bitcast(mybir.dt.bfloat16))

    qt_sb = singles.tile([D, H], F32)
    nc.sync.dma_start(out=qt_sb[:], in_=qt[:])
    id_sb = singles.tile([D, D], F32)
    nc.sync.dma_start(out=id_sb[:], in_=ident[:])
    pe_absorb(qt_sb[:, 0:1])
    pe_absorb(id_sb[:, 0:1])

    # one persistent L^T bank with 4 rotating 112-col slots: the slot-reuse
    # dependency (exp 4 groups back) is old enough that no wait is emitted
    ltall = ltp.tile([128, 4, CH * H], F32)

    for g in range(NG * repeat):
        g = g % NG
        chunk = chunks.tile([128, CH, DE], F32)
        nc.sync.dma_start(out=chunk[:], in_=xb[:, CH * g: CH * (g + 1), :])
        pe_absorb(chunk[:, 0, 0:1])

        ltb = ltall[:, g % 4, :]
        e_g = ep.tile([128, CH, M2], F32)
        # eps-fill the off-block weight columns: keeps mm2 output rows
        # 14..31 finite-garbage (no PSUM memset, no reciprocal infs)
        bc = id_sb[:, 0:1].to_broadcast([128, CH, M2 - E2])
        nc.scalar.activation(
            out=e_g[:, :, E2:M2], in_=bc,
            func=mybir.ActivationFunctionType.Copy, scale=0.0, bias=1e-3,
        )
        nc.scalar.activation(
            out=e_g[64:128, :, 0:H], in_=id_sb[0:64, 0:1].to_broadcast([64, CH, H]),
            func=mybir.ActivationFunctionType.Copy, scale=0.0, bias=0.0,
        )
        nc.scalar.activation(
            out=e_g[0:64, :, H:E2], in_=id_sb[0:64, 0:1].to_broadcast([64, CH, H]),
            func=mybir.ActivationFunctionType.Copy, scale=0.0, bias=0.0,
        )

        for q4 in range(CH // 4):
            xtb = xtp.tile([128, 512], F32)
            xsb = xts.tile([128, 512], F32)
            for i in range(4):
                c = q4 * 4 + i
                # transpose mode: 2 cycles/row for fp32 (vs 4 for plain mm)
                nc.tensor.matmul(
                    xtb[:, 128 * i: 128 * (i + 1)],
                    lhsT=chunk[:, c, 0:D],
                    rhs=id_sb[:],
                    start=True, stop=True,
                    is_transpose=True,
                )
            # relay engine fixed per pool slot (bufs=4) so each slot's
            # recycle dependency is a single, predictable engine
            if (g * 4 + q4) % 4 == 3:
                nc.scalar.copy(xsb[:], xtb[:])
            else:
                nc.vector.tensor_copy(xsb[:], xtb[:])
            pe_absorb(xsb[:, 0:1])
            for i in range(4):
                c = q4 * 4 + i
                nc.tensor.matmul(
                    ltb[:, H * c: H * (c + 1)],
                    lhsT=xsb[:, 128 * i: 128 * (i + 1)],
                    rhs=qt_sb[:],
                    start=True, stop=True,
                )

        lt3 = ltb.rearrange("p (c h) -> p c h", h=H)
        nc.scalar.activation(
            out=e_g[0:64, :, 0:H], in_=lt3[0:64],
            func=mybir.ActivationFunctionType.Exp,
        )
        nc.scalar.activation(
            out=e_g[64:128, :, H:E2], in_=lt3[64:128],
            func=mybir.ActivationFunctionType.Exp,
        )
        pe_absorb(e_g[:, 0, :])  # overlaps all five ACT writers of e_g

        # mm2: 8 pairs per PSUM bank; iterate col-groups fastest so the four
        # 32-col PE tiles run concurrently
        for half in range(2):
            ib = g * 2 + half
            ctxb = ctxp.tile([128, 2 * DE], F32)
            if _body.prev_st is not None:
                # absorb DVE progress so the ctx-bank recycle wait (recip/
                # scale of 3 banks ago) is observed before the matmuls
                pe_absorb(_body.prev_st[:, 0, 0:1])
            for k in range(2):
                for g2 in range(4):
                    c = half * 8 + 2 * g2 + k
                    nc.tensor.matmul(
                        ctxb[32 * g2: 32 * (g2 + 1), DE * k: DE * (k + 1)],
                        lhsT=e_g[:, c, :],
                        rhs=chunk[:, c, :],
                        start=True, stop=True,
                        tile_position=(0, 32 * g2),
                    )
            c3 = ctxb.rearrange("p (k e) -> p k e", e=DE)
            rr = rp.tile([128, 2], F32)
            nc.vector.reciprocal(rr[:], c3[:, :, D])
            st = stg.tile([128, 2, D], F32)
            for k in range(2):
                nc.vector.tensor_scalar_mul(
                    st[:, k, :], c3[:, k, 0:D], rr[:, k: k + 1],
                )
            _body.prev_st = st
            # one whole-bank store per ib; host reassembles (t,h,d) order
            nc.sync.dma_start(out=ob[ib], in_=st[:])


def _build(repeat=1):
    # Bacc (not bare Bass): its compile() runs move_matmul_waits_to_ldweights
    # + generate_event_semaphores, which legalize multi-wait instructions for
    # the TRN2 one-wait-per-instruction constraint.
    nc = bacc.Bacc("TRN2", target_bir_lowering=False, debug=False)
    xb = nc.dram_tensor("xb", [128, T // 2, D + 1], F32, kind="ExternalInput")
    qt = nc.dram_tensor("qt", [D, H], F32, kind="ExternalInput")
    ident = nc.dram_tensor("ident", [D, D], F32, kind="ExternalInput")
    # raw bank layout: (ib, 128 rows = [g2 x (7j+h | pad)], k, d);
    # host reassembles into (T, H, D)
    ob = nc.dram_tensor("ob", [T // 16, 128, 2, D], F32, kind="ExternalOutput")
    with tile.TileContext(nc) as tc:
        with ExitStack() as ctx:
            _body(ctx, tc, xb[:], qt[:], ident[:], ob[:], repeat=repeat)
    nc.compile()
    return nc


def get_nc(repeat=1):
    key = ("nc", repeat)
    if key not in _CACHE:
        _CACHE[key] = _build(repeat)
    return _CACHE[key]


def prep_input(others_b):
    """others[b] (N,T,D) -> ((j n), tp, D+1) with a trailing ones column."""
    x = np.empty((128, T // 2, D + 1), dtype=np.float32)
    v = others_b.reshape(N, T // 2, 2, D)          # n, tp, j, d
    x[:, :, D] = 1.0
    x[0:64, :, 0:D] = v[:, :, 0, :]                # j=0 rows 0..63  (n)
    x[64:128, :, 0:D] = v[:, :, 1, :]              # j=1 rows 64..127
    return x


def kernel(ego=None, others=None, queries=None, _trace=False, **_unused):
    others = np.asarray(others, dtype=np.float32)
    queries = np.asarray(queries, dtype=np.float32)
    scale = float(queries.shape[-1]) ** -0.5
    qt_scaled = np.ascontiguousarray(queries.T * scale).astype(np.float32)
    eye = np.eye(D, dtype=np.float32)

    nc = get_nc()
    in_maps = [
        {"xb": prep_input(others[b]), "qt": qt_scaled, "ident": eye}
        for b in range(B)
    ]
    res = run_bass_kernel_spmd(nc, in_maps, core_ids=list(range(B)), trace=_trace)
    _CACHE["last_results"] = res
    out = np.empty((B, T, H, D), dtype=np.float32)
    for b in range(B):
        out[b] = unpack_output(res.results[b]["ob"])
    return out


def unpack_output(ob_raw):
    """(T/16, 128, 2, D) bank layout -> (T, H, D); t = 16 ib + 4 g2 + 2k + j."""
    s = ob_raw.reshape(T // 16, 4, 32, 2, D)[:, :, : 2 * H]
    s = s.reshape(T // 16, 4, 2, H, 2, D)          # ib, g2, j, h, k, d
    return np.ascontiguousarray(
        s.transpose(0, 1, 4, 2, 3, 5).reshape(T, H, D)
    )



# revision 3
# speedup vs baseline: 2.3728x; 2.3728x over previous
"""MultiHeadPool Trainium2 kernel (bf16, transpose-free).

Per-core computation (batch b of 8, one per NeuronCore):
  X = others[b]          (N=64, T=512, D=128)
  L = X . qT * scale     contraction over d   -> (T, H, N) logits
  W = softmax_n(L)
  ctx = W . X            contraction over n   -> (T, H, D)

The kernel is PE-instruction-issue bound (~70-80ns/instr), so the design
minimizes Tensor-engine instruction count:
  - bf16 operands: 1 LDWEIGHTS + 1 MATMUL per logical matmul (fp32 emits
    2+2), and half the HBM traffic.
  - The host sends TWO layouts of X: xd[d, (t n)] (d-major) and
    xjn[(j n), tp, d'] (pair-major, d'=D plus a ones column). mm1 loads
    its stationary X^T_c directly from xd -- no PE transposes, no
    PSUM->SBUF relay copies.
  - Block-diagonal zero padding of the E tiles is written ONCE into a
    persistent 2-slot tile; per-chunk exp writes only the diagonal
    blocks.

Per t-pair c (rows = 64j+n, two timesteps t=2c+j):
  mm1: L_c[(jn), h] = xd[:,128c:128c+128].T @ qt      (PSUM, f=7)
  exp: E[(jn), c, j'*7+h] = exp(L) on the j==j' diagonal blocks (bf16)
  mm2: ctx-pair = E_c.T @ xjn_c -> (32, 129) at col-group 32*g2;
       col 128 = softmax denominators; 4 col-groups packed on the PE
  DVE: reciprocal + broadcast scale; DMA out (host reassembles order)
"""

import sys

for p in ("/opt/trn_rl_repo", "/root/.axon_site/_ro/trn_rl_repo"):
    if p not in sys.path:
        sys.path.append(p)

from contextlib import ExitStack

import numpy as np
import ml_dtypes

import concourse.bacc as bacc
import concourse.bass as bass
import concourse.tile as tile
from concourse import mybir
from concourse.bass_utils import run_bass_kernel_spmd

B, N, T, D, H = 8, 64, 512, 128, 7
CH = 16               # t-pairs per chunk
NG = (T // 2) // CH   # 16 chunks per batch
F32 = mybir.dt.float32
BF16 = mybir.dt.bfloat16
BF16_NP = ml_dtypes.bfloat16

_CACHE = {}


def _body(ctx, tc, xd, xjn, qt, ob, repeat=1):
    nc = tc.nc
    E2 = 2 * H            # 14 data cols; pad to 32 for full g2 row groups
    M2 = 32
    DE = D + 1            # 129
    NSLOT = 2             # e-tile slots (manual rotation, zeros persist)

    singles = ctx.enter_context(tc.tile_pool(name="singles", bufs=1))
    ltp = ctx.enter_context(tc.tile_pool(name="ltp", bufs=1, space="PSUM"))
    ctxp = ctx.enter_context(tc.tile_pool(name="ctxp", bufs=3, space="PSUM"))
    rp = ctx.enter_context(tc.tile_pool(name="rp", bufs=4))
    stg = ctx.enter_context(tc.tile_pool(name="stg", bufs=3))

    qt_sb = singles.tile([D, H], BF16)
    nc.sync.dma_start(out=qt_sb[:], in_=qt[:])

    # persistent SBUF-resident inputs; chunked DMAs all issued up front so
    # the DMA engine streams ahead of compute
    xd_sb = singles.tile([128, NG, CH * 128], BF16)
    xjn_sb = singles.tile([128, T // 2, DE], BF16)
    for g in range(NG):
        nc.sync.dma_start(out=xd_sb[:, g, :], in_=xd[:, g, :])
        nc.sync.dma_start(out=xjn_sb[:, CH * g: CH * (g + 1), :],
                          in_=xjn[:, CH * g: CH * (g + 1), :])

    # E tiles: [jn, slot, c, m]; m in 0:7 is the j=0 block, 7:14 the j=1
    # block, 14:32 zero pad (keeps mm2 output a full 32-row group). The
    # off-diagonal + pad zeros are written once and never touched again.
    e_all = singles.tile([128, NSLOT, CH, M2], BF16)
    nc.vector.memset(e_all[:], 0.0)

    # one persistent L bank with 4 rotating slots (448 fp32 <= 1 bank);
    # slot-reuse deps are 4 chunks old -> elided
    ltall = ltp.tile([128, 4, CH, H], F32)

    for g in range(NG * repeat):
        g = g % NG
        ltb = ltall[:, g % 4]
        for i in range(CH):
            nc.tensor.matmul(
                ltb[:, i, :],
                lhsT=xd_sb[:, g, 128 * i: 128 * (i + 1)],
                rhs=qt_sb[:],
                start=True, stop=True,
            )

        e_g = e_all[:, g % NSLOT]
        nc.scalar.activation(
            out=e_g[0:64, :, 0:H], in_=ltb[0:64],
            func=mybir.ActivationFunctionType.Exp,
        )
        nc.scalar.activation(
            out=e_g[64:128, :, H:E2], in_=ltb[64:128],
            func=mybir.ActivationFunctionType.Exp,
        )

        # mm2: 8 pairs per PSUM bank; col-groups iterate fastest so the
        # four 32-col PE tiles run concurrently
        for half in range(2):
            ib = g * 2 + half
            ctxb = ctxp.tile([128, 2, DE], F32)
            for k in range(2):
                for g2 in range(4):
                    c = half * 8 + 2 * g2 + k
                    nc.tensor.matmul(
                        ctxb[32 * g2: 32 * (g2 + 1), k, :],
                        lhsT=e_g[:, c, :],
                        rhs=xjn_sb[:, CH * g + c, :],
                        start=True, stop=True,
                        tile_position=(0, 32 * g2),
                    )
            rr = rp.tile([128, 2], F32)
            nc.vector.reciprocal(rr[:], ctxb[:, :, D])
            st = stg.tile([128, 2, D], F32)
            for k in range(2):
                nc.vector.tensor_scalar_mul(
                    st[:, k, :], ctxb[:, k, 0:D], rr[:, k: k + 1],
                )
            # one whole-bank store per ib; host reassembles (t,h,d) order
            nc.sync.dma_start(out=ob[ib], in_=st[:])


def _build(repeat=1):
    # Bacc (not bare Bass): its compile() runs move_matmul_waits_to_ldweights
    # + generate_event_semaphores, which legalize multi-wait instructions for
    # the TRN2 one-wait-per-instruction constraint.
    nc = bacc.Bacc("TRN2", target_bir_lowering=False, debug=False)
    xd = nc.dram_tensor("xd", [128, NG, CH * 128], BF16, kind="ExternalInput")
    xjn = nc.dram_tensor("xjn", [128, T // 2, D + 1], BF16, kind="ExternalInput")
    qt = nc.dram_tensor("qt", [D, H], BF16, kind="ExternalInput")
    # raw bank layout: (ib, 128 rows = [g2 x (7j+h | pad)], k, d);
    # host reassembles into (T, H, D)
    ob = nc.dram_tensor("ob", [T // 16, 128, 2, D], F32, kind="ExternalOutput")
    with tile.TileContext(nc) as tc:
        with ExitStack() as ctx:
            _body(ctx, tc, xd[:], xjn[:], qt[:], ob[:], repeat=repeat)
    nc.compile()
    return nc


def get_nc(repeat=1):
    key = ("nc", repeat)
    if key not in _CACHE:
        _CACHE[key] = _build(repeat)
    return _CACHE[key]


def prep_inputs(others_b):
    """others[b] (N,T,D) -> (xd, xjn) bf16 layouts.

    xd[d, g, 128c'+64j+n] = others[n, t, d] for t = 2(16g+c')+j
    xjn[64j+n, tp, d] = others[n, 2 tp + j, d], with a trailing ones col.
    """
    xd = np.ascontiguousarray(
        others_b.transpose(2, 1, 0)                 # d, t, n
    ).reshape(128, NG, CH * 128).astype(BF16_NP)
    xjn = np.empty((128, T // 2, D + 1), dtype=BF16_NP)
    v = others_b.reshape(N, T // 2, 2, D)           # n, tp, j, d
    xjn[:, :, D] = 1.0
    xjn[0:64, :, 0:D] = v[:, :, 0, :]
    xjn[64:128, :, 0:D] = v[:, :, 1, :]
    return xd, xjn


def kernel(ego=None, others=None, queries=None, _trace=False, **_unused):
    others = np.asarray(others, dtype=np.float32)
    queries = np.asarray(queries, dtype=np.float32)
    scale = float(queries.shape[-1]) ** -0.5
    qt_scaled = np.ascontiguousarray(queries.T * scale).astype(BF16_NP)

    nc = get_nc()
    in_maps = []
    for b in range(B):
        xd, xjn = prep_inputs(others[b])
        in_maps.append({"xd": xd, "xjn": xjn, "qt": qt_scaled})
    res = run_bass_kernel_spmd(nc, in_maps, core_ids=list(range(B)), trace=_trace)
    _CACHE["last_results"] = res
    out = np.empty((B, T, H, D), dtype=np.float32)
    for b in range(B):
        out[b] = unpack_output(res.results[b]["ob"])
    return out


def unpack_output(ob_raw):
    """(T/16, 128, 2, D) bank layout -> (T, H, D); t = 16 ib + 4 g2 + 2k + j."""
    s = ob_raw.reshape(T // 16, 4, 32, 2, D)[:, :, : 2 * H]
    s = s.reshape(T // 16, 4, 2, H, 2, D)          # ib, g2, j, h, k, d
    return np.ascontiguousarray(
        s.transpose(0, 1, 4, 2, 3, 5).reshape(T, H, D)
    )


# revision 8
# speedup vs baseline: 2.4879x; 1.0485x over previous
"""MultiHeadPool Trainium2 kernel (bf16, transpose-free).

Per-core computation (batch b of 8, one per NeuronCore):
  X = others[b]          (N=64, T=512, D=128)
  L = X . qT * scale     contraction over d   -> (T, H, N) logits
  W = softmax_n(L)
  ctx = W . X            contraction over n   -> (T, H, D)

The kernel is PE-instruction-issue bound (~70-80ns/instr), so the design
minimizes Tensor-engine instruction count:
  - bf16 operands: 1 LDWEIGHTS + 1 MATMUL per logical matmul (fp32 emits
    2+2), and half the HBM traffic.
  - The host sends TWO layouts of X: xd[d, (t n)] (d-major) and
    xjn[(j n), tp, d'] (pair-major, d'=D plus a ones column). mm1 loads
    its stationary X^T_c directly from xd -- no PE transposes, no
    PSUM->SBUF relay copies.
  - Block-diagonal zero padding of the E tiles is written ONCE into a
    persistent 2-slot tile; per-chunk exp writes only the diagonal
    blocks.

Per t-pair c (rows = 64j+n, two timesteps t=2c+j):
  mm1: L_c[(jn), h] = xd[:,128c:128c+128].T @ qt      (PSUM, f=7)
  exp: E[(jn), c, j'*7+h] = exp(L) on the j==j' diagonal blocks (bf16)
  mm2: ctx-pair = E_c.T @ xjn_c -> (32, 129) at col-group 32*g2;
       col 128 = softmax denominators; 4 col-groups packed on the PE
  DVE: reciprocal + broadcast scale; DMA out (host reassembles order)
"""

import sys

for p in ("/opt/trn_rl_repo", "/root/.axon_site/_ro/trn_rl_repo"):
    if p not in sys.path:
        sys.path.append(p)

from contextlib import ExitStack

import numpy as np
import ml_dtypes

import concourse.bacc as bacc
import concourse.bass as bass
import concourse.tile as tile
from concourse import mybir
from concourse.bass_utils import run_bass_kernel_spmd

B, N, T, D, H = 8, 64, 512, 128, 7
CH = 16               # t-pairs per chunk
NG = (T // 2) // CH   # 16 chunks per batch
F32 = mybir.dt.float32
BF16 = mybir.dt.bfloat16
BF16_NP = ml_dtypes.bfloat16

_CACHE = {}


def _body(ctx, tc, xd, xjn, qt, ob, repeat=1):
    nc = tc.nc
    E2 = 2 * H            # 14 data cols; pad to 32 for full g2 row groups
    M2 = 32
    DE = D + 1            # 129
    NSLOT = 2             # e-tile slots (manual rotation, zeros persist)

    singles = ctx.enter_context(tc.tile_pool(name="singles", bufs=1))
    ltp = ctx.enter_context(tc.tile_pool(name="ltp", bufs=1, space="PSUM"))
    ctxp = ctx.enter_context(tc.tile_pool(name="ctxp", bufs=3, space="PSUM"))
    rp = ctx.enter_context(tc.tile_pool(name="rp", bufs=4))
    stg = ctx.enter_context(tc.tile_pool(name="stg", bufs=3))

    qt_sb = singles.tile([D, H], BF16)
    nc.sync.dma_start(out=qt_sb[:], in_=qt[:])

    # persistent SBUF-resident inputs; chunk DMAs are issued with a small
    # prefetch depth so output DMAs (scalar queue) interleave with the
    # input stream instead of queueing behind all of it
    xd_sb = singles.tile([128, NG, CH * 128], BF16)
    xjn_sb = singles.tile([128, T // 2, DE], BF16)
    PF = 3

    def fetch(g):
        nc.sync.dma_start(out=xd_sb[:, g, :], in_=xd[:, g, :])
        nc.sync.dma_start(out=xjn_sb[:, CH * g: CH * (g + 1), :],
                          in_=xjn[:, CH * g: CH * (g + 1), :])

    for g in range(min(PF, NG)):
        fetch(g)

    # E tiles: [jn, slot, c, m]; m in 0:7 is the j=0 block, 7:14 the j=1
    # block, 14:32 zero pad (keeps mm2 output a full 32-row group). The
    # off-diagonal + pad zeros are written once and never touched again.
    e_all = singles.tile([128, NSLOT, CH, M2], BF16)
    nc.vector.memset(e_all[:], 0.0)

    # one persistent L bank with 4 rotating slots (448 fp32 <= 1 bank);
    # slot-reuse deps are 4 chunks old -> elided
    ltall = ltp.tile([128, 4, CH, H], F32)

    for g in range(NG * repeat):
        g = g % NG
        if g + PF < NG:
            fetch(g + PF)
        ltb = ltall[:, g % 4]
        for i in range(CH):
            nc.tensor.matmul(
                ltb[:, i, :],
                lhsT=xd_sb[:, g, 128 * i: 128 * (i + 1)],
                rhs=qt_sb[:],
                start=True, stop=True,
            )

        e_g = e_all[:, g % NSLOT]
        nc.scalar.activation(
            out=e_g[0:64, :, 0:H], in_=ltb[0:64],
            func=mybir.ActivationFunctionType.Exp,
        )
        nc.scalar.activation(
            out=e_g[64:128, :, H:E2], in_=ltb[64:128],
            func=mybir.ActivationFunctionType.Exp,
        )

        # mm2: 8 pairs per PSUM bank; col-groups iterate fastest so the
        # four 32-col PE tiles run concurrently
        for half in range(2):
            ib = g * 2 + half
            ctxb = ctxp.tile([128, 2, DE], F32)
            for k in range(2):
                for g2 in range(4):
                    c = half * 8 + 2 * g2 + k
                    nc.tensor.matmul(
                        ctxb[32 * g2: 32 * (g2 + 1), k, :],
                        lhsT=e_g[:, c, :],
                        rhs=xjn_sb[:, CH * g + c, :],
                        start=True, stop=True,
                        tile_position=(0, 32 * g2),
                    )
            rr = rp.tile([128, 2], F32)
            nc.vector.reciprocal(rr[:], ctxb[:, :, D])
            st = stg.tile([128, 2, D], BF16)
            for k in range(2):
                nc.vector.tensor_scalar_mul(
                    st[:, k, :], ctxb[:, k, 0:D], rr[:, k: k + 1],
                )
            # one whole-bank store per ib on the scalar DMA queue so the
            # output stream interleaves with (not behind) the input stream
            nc.scalar.dma_start(out=ob[ib], in_=st[:])


def _build(repeat=1):
    # Bacc (not bare Bass): its compile() runs move_matmul_waits_to_ldweights
    # + generate_event_semaphores, which legalize multi-wait instructions for
    # the TRN2 one-wait-per-instruction constraint.
    nc = bacc.Bacc("TRN2", target_bir_lowering=False, debug=False)
    xd = nc.dram_tensor("xd", [128, NG, CH * 128], BF16, kind="ExternalInput")
    xjn = nc.dram_tensor("xjn", [128, T // 2, D + 1], BF16, kind="ExternalInput")
    qt = nc.dram_tensor("qt", [D, H], BF16, kind="ExternalInput")
    # raw bank layout: (ib, 128 rows = [g2 x (7j+h | pad)], k, d);
    # host reassembles into (T, H, D)
    ob = nc.dram_tensor("ob", [T // 16, 128, 2, D], BF16, kind="ExternalOutput")
    with tile.TileContext(nc) as tc:
        with ExitStack() as ctx:
            _body(ctx, tc, xd[:], xjn[:], qt[:], ob[:], repeat=repeat)
    nc.compile()
    return nc


def get_nc(repeat=1):
    key = ("nc", repeat)
    if key not in _CACHE:
        _CACHE[key] = _build(repeat)
    return _CACHE[key]


def prep_inputs(others_b):
    """others[b] (N,T,D) -> (xd, xjn) bf16 layouts.

    xd[d, g, 128c'+64j+n] = others[n, t, d] for t = 2(16g+c')+j
    xjn[64j+n, tp, d] = others[n, 2 tp + j, d], with a trailing ones col.
    """
    xd = np.ascontiguousarray(
        others_b.transpose(2, 1, 0)                 # d, t, n
    ).reshape(128, NG, CH * 128).astype(BF16_NP)
    xjn = np.empty((128, T // 2, D + 1), dtype=BF16_NP)
    v = others_b.reshape(N, T // 2, 2, D)           # n, tp, j, d
    xjn[:, :, D] = 1.0
    xjn[0:64, :, 0:D] = v[:, :, 0, :]
    xjn[64:128, :, 0:D] = v[:, :, 1, :]
    return xd, xjn


def kernel(ego=None, others=None, queries=None, _trace=False, **_unused):
    others = np.asarray(others, dtype=np.float32)
    queries = np.asarray(queries, dtype=np.float32)
    scale = float(queries.shape[-1]) ** -0.5
    qt_scaled = np.ascontiguousarray(queries.T * scale).astype(BF16_NP)

    nc = get_nc()
    in_maps = []
    for b in range(B):
        xd, xjn = prep_inputs(others[b])
        in_maps.append({"xd": xd, "xjn": xjn, "qt": qt_scaled})
    res = run_bass_kernel_spmd(nc, in_maps, core_ids=list(range(B)), trace=_trace)
    _CACHE["last_results"] = res
    out = np.empty((B, T, H, D), dtype=np.float32)
    for b in range(B):
        out[b] = unpack_output(res.results[b]["ob"])
    return out


def unpack_output(ob_raw):
    """(T/16, 128, 2, D) bank layout -> (T, H, D); t = 16 ib + 4 g2 + 2k + j."""
    s = ob_raw.reshape(T // 16, 4, 32, 2, D)[:, :, : 2 * H]
    s = s.reshape(T // 16, 4, 2, H, 2, D)          # ib, g2, j, h, k, d
    return np.ascontiguousarray(
        s.transpose(0, 1, 4, 2, 3, 5).reshape(T, H, D).astype(np.float32)
    )
